# revision 1
# baseline (speedup 1.0000x reference)
"""Trainium2 Bass kernel for nn_Convolution_84172769067729 (e3nn-style GNN conv).

Graph/edge parallelism over 8 NeuronCores:
  - Node space padded to 50176 = 8*49*128; core k owns dst range
    [k*6272,(k+1)*6272). Host buckets edges by destination node-tile (128
    nodes), pads each (core,bucket) to a uniform B_max tiles of 128 edges
    (zero-embedding pad edges contribute exactly zero), and pre-permutes
    all per-edge arrays into that order.
  - Stage B: y = lin1(x*attr) into an HBM table (partition-major row
    mapping so table writes are large contiguous runs); self-connection s
    for the own range. Host folds all norm factors / c_s / c_x /
    1/sqrt(NUM_NEIGHBORS) into the weights.
  - Stage C per bucket: indirect-DMA gather y[src] (f32->bf16 cast in
    DMA), radial MLP (PE + ACT silu, bf16), tensor product as bf16 DVE
    slab ops, scatter-add via one-hot matmuls accumulated in PSUM, lin2 +
    combine, output written once as a [128, 6272] partition-major slab.
Host unshards: reorder columns (d-major -> u-major), un-permute rows.
"""
import math
import numpy as np
import ml_dtypes

MUL = 32
NCORES = 8
NODE_TILE = 128
NT_PER_CORE = 49
NODES_PER_CORE = NT_PER_CORE * NODE_TILE      # 6272
NN = NCORES * NODES_PER_CORE                  # 50176
NTILES = NN // 128                            # 392
NB = 10
NHID = 100

_BF16 = np.dtype(ml_dtypes.bfloat16)


def _prep(inputs):
    node_input = np.asarray(inputs["node_input"], np.float32)
    node_attr = np.asarray(inputs["node_attr"], np.float32)
    edge_src = np.asarray(inputs["edge_src"], np.int32)
    edge_dst = np.asarray(inputs["edge_dst"], np.int32)
    edge_attr = np.asarray(inputs["edge_attr"], np.float32)
    elemb = np.asarray(inputs["edge_length_embedded"], np.float32)
    N = node_input.shape[0]

    c_s = math.sin(math.pi / 8.0)
    c_x = math.cos(math.pi / 8.0)
    Wl10 = np.asarray(inputs["W_l10"], np.float32) / math.sqrt(MUL)
    Wl11 = np.asarray(inputs["W_l11"], np.float32) / math.sqrt(MUL)
    Wsc0 = np.asarray(inputs["W_sc0"], np.float32) * (c_s / math.sqrt(MUL))
    Wsc1 = np.asarray(inputs["W_sc1"], np.float32) * (c_s / math.sqrt(MUL))
    Wfc1 = np.asarray(inputs["W_fc1"], np.float32) / math.sqrt(NB)
    Wfc2 = (np.asarray(inputs["W_fc2"], np.float32) / math.sqrt(NHID)).copy()
    Wfc2[:, 3 * MUL:] /= math.sqrt(3.0)
    zf = c_x / math.sqrt(2 * MUL) / math.sqrt(16.0)
    Wl20 = np.asarray(inputs["W_l20"], np.float32) * zf
    Wl21 = np.asarray(inputs["W_l21"], np.float32) * zf
    # F column layout interleaves mid1a_d/mid1b_d as 64-wide [C_d|D_d] pairs.
    # lin2 weights zero-padded to K=128 so every z matmul reads a full
    # base-partition-0 aggT half (base!=0 operands crash the PE).
    Wz0 = np.zeros((128, 32), np.float32)
    Wz0[0:64] = Wl20
    WzL = np.zeros((128, 32), np.float32)
    WzL[0:64] = Wl21
    WzH = np.zeros((128, 32), np.float32)
    WzH[64:128] = Wl21

    Wbig = np.zeros((128, 128), np.float32)
    Wscbig = np.zeros((128, 128), np.float32)
    Wbig[0:32, 0:32] = Wl10
    Wscbig[0:32, 0:32] = Wsc0
    for d in range(3):
        sl = slice(32 + 32 * d, 64 + 32 * d)
        Wbig[sl, sl] = Wl11
        Wscbig[sl, sl] = Wsc1

    x0 = node_input[:, :MUL] * node_attr
    x1 = node_input[:, MUL:].reshape(N, MUL, 3) * node_attr[:, :, None]
    xdT = np.zeros((128, NN), np.float32)
    xdT[0:32, :N] = x0.T
    for d in range(3):
        xdT[32 + 32 * d:64 + 32 * d, :N] = x1[:, :, d].T

    attr = np.zeros(NN, np.float32)
    attr[:N] = node_attr[:, 0]
    attrP = attr.reshape(NTILES, 128).T.copy()      # [128, 392]

    core_of = edge_dst // NODES_PER_CORE
    bucket_of = (edge_dst % NODES_PER_CORE) // NODE_TILE
    counts = np.zeros((NCORES, NT_PER_CORE), np.int64)
    np.add.at(counts, (core_of, bucket_of), 1)
    B_max = int(np.ceil(counts.max() / 128))
    T = NT_PER_CORE * B_max
    Epc = T * 128

    order = np.argsort(edge_dst, kind="stable")
    es, ed = edge_src[order], edge_dst[order]
    ea_s, eb_s = edge_attr[order], elemb[order]
    flat_bucket = (edge_dst[order] // NODE_TILE)
    starts = np.searchsorted(flat_bucket, np.arange(NCORES * NT_PER_CORE))
    ends = np.searchsorted(flat_bucket, np.arange(NCORES * NT_PER_CORE) + 1)

    cores = []
    for k in range(NCORES):
        srcidx = np.zeros(Epc, np.int32)
        dstloc = np.zeros(Epc, np.float32)
        ea = np.zeros((Epc, 4), np.float32)
        eb = np.zeros((Epc, NB), np.float32)
        for b in range(NT_PER_CORE):
            g = k * NT_PER_CORE + b
            i0, i1 = starts[g], ends[g]
            n = i1 - i0
            o = b * B_max * 128
            srcidx[o:o + n] = es[i0:i1]
            dstloc[o:o + n] = (ed[i0:i1] % NODE_TILE).astype(np.float32)
            ea[o:o + n] = ea_s[i0:i1]
            eb[o:o + n] = eb_s[i0:i1]
        # partition-major ytab row mapping: node n -> row (n%128)*392 + n//128
        srcmap = (srcidx % 128) * NTILES + srcidx // 128
        cores.append({
            "srcidx": np.ascontiguousarray(srcmap.reshape(T, 128).T),
            "dstloc": np.ascontiguousarray(dstloc.reshape(T, 128).T).astype(_BF16),
            "eaP": np.ascontiguousarray(
                ea.reshape(T, 128, 4).transpose(1, 0, 2).reshape(128, T * 4)
            ).astype(_BF16),
            "ebT": np.ascontiguousarray(eb.T).astype(_BF16),
        })
    shared = {
        "xdT": xdT, "attrP": attrP, "Wbig": Wbig, "Wscbig": Wscbig,
        "Wfc1": Wfc1.astype(_BF16), "Wfc2": Wfc2.astype(_BF16),
        "Wz0": Wz0, "WzL": WzL, "WzH": WzH,
    }
    meta = {"B_max": B_max, "T": T, "Epc": Epc, "N": N}
    return cores, shared, meta


def _build_nc(meta, reps=1, stage=10):
    import concourse.bacc as bacc
    import concourse.bass as bass
    import concourse.mybir as mybir
    import concourse.tile as tile

    f32 = mybir.dt.float32
    bf16 = mybir.dt.bfloat16
    i32 = mybir.dt.int32
    i16 = mybir.dt.int16
    AF = mybir.ActivationFunctionType
    OP = mybir.AluOpType
    B = meta["B_max"]
    T = meta["T"]
    Epc = meta["Epc"]
    EB = B * 128

    nc = bacc.Bacc("TRN2", debug=False, num_devices=NCORES)

    xdT_d = nc.dram_tensor("xdT", [128, NN], f32, kind="ExternalInput")
    xdTo_d = nc.dram_tensor("xdT_own", [128, NODES_PER_CORE], f32, kind="ExternalInput")
    attrP_d = nc.dram_tensor("attrP", [128, NT_PER_CORE], f32, kind="ExternalInput")
    Wbig_d = nc.dram_tensor("Wbig", [128, 128], f32, kind="ExternalInput")
    Wscbig_d = nc.dram_tensor("Wscbig", [128, 128], f32, kind="ExternalInput")
    Wfc1_d = nc.dram_tensor("Wfc1", [NB, NHID], bf16, kind="ExternalInput")
    Wfc2_d = nc.dram_tensor("Wfc2", [NHID, 128], bf16, kind="ExternalInput")
    Wz0_d = nc.dram_tensor("Wz0", [128, 32], f32, kind="ExternalInput")
    WzL_d = nc.dram_tensor("WzL", [128, 32], f32, kind="ExternalInput")
    WzH_d = nc.dram_tensor("WzH", [128, 32], f32, kind="ExternalInput")
    src_d = nc.dram_tensor("srcidx", [128, T], i32, kind="ExternalInput")
    dst_d = nc.dram_tensor("dstloc", [128, T], bf16, kind="ExternalInput")
    eaP_d = nc.dram_tensor("eaP", [128, T * 4], bf16, kind="ExternalInput")
    ebT_d = nc.dram_tensor("ebT", [NB, Epc], bf16, kind="ExternalInput")
    ytab = nc.dram_tensor("ytab", [NN, 128], f32)
    out_d = nc.dram_tensor("outp", [128, NT_PER_CORE * 128], f32, kind="ExternalOutput")

    with tile.TileContext(nc) as tc:
        with (
            tc.tile_pool(name="const", bufs=1) as cpool,
            tc.tile_pool(name="xslab", bufs=2) as xpool,
            tc.tile_pool(name="ysl", bufs=2) as ypool,
            tc.tile_pool(name="persist", bufs=1) as ppool,
            tc.tile_pool(name="eb", bufs=2) as ebpool,
            tc.tile_pool(name="xs", bufs=2) as xspool,
            tc.tile_pool(name="work", bufs=2) as wpool,
            tc.tile_pool(name="small", bufs=2) as smpool,
            tc.tile_pool(name="ps_big", bufs=2, space="PSUM") as ps_big,
            tc.tile_pool(name="ps_agg", bufs=2, space="PSUM") as ps_agg,
            tc.tile_pool(name="ps_z", bufs=2, space="PSUM") as ps_z,
        ):
            Wbig_sb = cpool.tile([128, 128], f32)
            nc.sync.dma_start(Wbig_sb[:], Wbig_d[:])
            Wscbig_sb = cpool.tile([128, 128], f32)
            nc.sync.dma_start(Wscbig_sb[:], Wscbig_d[:])
            Wfc1_sb = cpool.tile([NB, NHID], bf16)
            nc.sync.dma_start(Wfc1_sb[:], Wfc1_d[:])
            Wfc2_sb = cpool.tile([NHID, 128], bf16)
            nc.sync.dma_start(Wfc2_sb[:], Wfc2_d[:])
            Wz0_sb = cpool.tile([128, 32], f32)
            nc.sync.dma_start(Wz0_sb[:], Wz0_d[:])
            WzL_sb = cpool.tile([128, 32], f32)
            nc.sync.dma_start(WzL_sb[:], WzL_d[:])
            WzH_sb = cpool.tile([128, 32], f32)
            nc.sync.dma_start(WzH_sb[:], WzH_d[:])
            attrP_sb = cpool.tile([128, NT_PER_CORE], f32)
            nc.sync.dma_start(attrP_sb[:], attrP_d[:])
            src_sb = cpool.tile([128, T], i32)
            nc.sync.dma_start(src_sb[:], src_d[:])
            dst_sb = cpool.tile([128, T], bf16)
            nc.sync.dma_start(dst_sb[:], dst_d[:])
            eaP_sb = cpool.tile([128, T * 4], bf16)
            nc.sync.dma_start(eaP_sb[:], eaP_d[:])

            iota_i = cpool.tile([128, EB], i16)
            nc.gpsimd.iota(iota_i[:].rearrange("p (t j) -> p t j", j=128),
                           pattern=[[0, B], [1, 128]], channel_multiplier=0)
            iota_sb = cpool.tile([128, EB], bf16)
            nc.vector.tensor_copy(iota_sb[:], iota_i[:])

            iden_i = cpool.tile([128, 128], i16)
            nc.gpsimd.iota(iden_i[:], pattern=[[1, 128]], channel_multiplier=0)
            iden_t = cpool.tile([128, 128], f32)
            nc.vector.tensor_copy(iden_t[:], iden_i[:])
            pidx_i = cpool.tile([128, 1], i16)
            nc.gpsimd.iota(pidx_i[:], pattern=[[0, 1]], channel_multiplier=1)
            pidx_f = cpool.tile([128, 1], f32)
            nc.vector.tensor_copy(pidx_f[:], pidx_i[:])
            iden_f = cpool.tile([128, 128], f32)
            nc.vector.tensor_scalar(out=iden_f[:], in0=iden_t[:],
                                    scalar1=pidx_f[:, 0:1], scalar2=None,
                                    op0=OP.is_equal)

            s_sb = ppool.tile([128, NODES_PER_CORE], f32)
            o_sb = ppool.tile([128, NODES_PER_CORE], f32)
            if stage < 10:
                nc.vector.memset(o_sb[:], 0)
            ytv = ytab[:].rearrange("(p t) f -> p t f", t=NTILES)

            def body():
                # ---------- stage B ----------
                SL = 16
                for s0 in range(0, NTILES, SL):
                    nsl = min(SL, NTILES - s0)
                    xsl = xpool.tile([128, SL * 128], f32, tag="xsl")
                    nc.sync.dma_start(xsl[:, :nsl * 128],
                                      xdT_d[:, s0 * 128:(s0 + nsl) * 128])
                    ysl = ypool.tile([128, SL * 128], f32, tag="ysl")
                    for g in range(0, nsl, 4):
                        nq = min(4, nsl - g)
                        yps = ps_big.tile([128, 512], f32, tag="big")
                        for q in range(nq):
                            nc.tensor.matmul(
                                out=yps[:, q * 128:(q + 1) * 128],
                                lhsT=xsl[:, (g + q) * 128:(g + q + 1) * 128],
                                rhs=Wbig_sb[:], start=True, stop=True)
                        nc.scalar.activation(ysl[:, g * 128:(g + nq) * 128],
                                             yps[:, :nq * 128], AF.Copy)
                    nc.sync.dma_start(
                        ytv[:, s0:s0 + nsl, :],
                        ysl[:, :nsl * 128].rearrange("p (q f) -> p q f", f=128))
                for b0 in range(0, NT_PER_CORE, 4):
                    nb4 = min(4, NT_PER_CORE - b0)
                    xso = xpool.tile([128, 4 * 128], f32, tag="xso")
                    nc.sync.dma_start(xso[:, :nb4 * 128],
                                      xdTo_d[:, b0 * 128:(b0 + nb4) * 128])
                    sps = ps_big.tile([128, 512], f32, tag="big")
                    for q in range(nb4):
                        nc.tensor.matmul(out=sps[:, q * 128:(q + 1) * 128],
                                         lhsT=xso[:, q * 128:(q + 1) * 128],
                                         rhs=Wscbig_sb[:], start=True, stop=True)
                    nc.scalar.activation(s_sb[:, b0 * 128:(b0 + nb4) * 128],
                                         sps[:, :nb4 * 128], AF.Copy)

                # ---------- stage C ----------
                for b in range(NT_PER_CORE if stage >= 2 else 0):
                    e0 = b * EB
                    ebsl = ebpool.tile([NB, EB], bf16, tag="ebsl")
                    nc.sync.dma_start(ebsl[:], ebT_d[:, e0:e0 + EB])
                    xs = xspool.tile([128, EB], bf16, tag="xs")
                    for t in range(B):
                        nc.gpsimd.indirect_dma_start(
                            out=xs[:, t * 128:(t + 1) * 128],
                            out_offset=None,
                            in_=ytab[:],
                            in_offset=bass.IndirectOffsetOnAxis(
                                ap=src_sb[:, b * B + t:b * B + t + 1], axis=0),
                        )
                    if stage < 3:
                        continue
                    hT = wpool.tile([NHID, EB], bf16, tag="hT")
                    for c0 in range(0, EB, 512):
                        cw = min(512, EB - c0)
                        hps = ps_big.tile([NHID, 512], f32, tag="big")
                        nc.tensor.matmul(out=hps[:, :cw], lhsT=Wfc1_sb[:],
                                         rhs=ebsl[:, c0:c0 + cw],
                                         start=True, stop=True)
                        nc.scalar.activation(hT[:, c0:c0 + cw], hps[:, :cw],
                                             AF.Silu)
                    if stage < 4:
                        continue
                    wsl = wpool.tile([128, EB], bf16, tag="wsl")
                    for t0 in range(0, B, 4):
                        nt4 = min(4, B - t0)
                        wps = ps_big.tile([128, 512], f32, tag="big")
                        for q in range(nt4):
                            t = t0 + q
                            nc.tensor.matmul(out=wps[:, q * 128:(q + 1) * 128],
                                             lhsT=hT[:, t * 128:(t + 1) * 128],
                                             rhs=Wfc2_sb[:], start=True, stop=True)
                        nc.vector.tensor_copy(wsl[:, t0 * 128:(t0 + nt4) * 128],
                                              wps[:, :nt4 * 128])

                    if stage < 5:
                        continue
                    xs3 = xs[:].rearrange("p (t f) -> p t f", f=128)
                    w3v = wsl[:].rearrange("p (t f) -> p t f", f=128)
                    ea3 = eaP_sb[:, b * B * 4:(b + 1) * B * 4].rearrange(
                        "p (t j) -> p t j", j=4)
                    F = wpool.tile([128, B * 256], bf16, tag="F")
                    F3 = F[:].rearrange("p (t f) -> p t f", f=256)
                    wa = smpool.tile([128, B * 32], bf16, tag="wa")
                    wa3 = wa[:].rearrange("p (t f) -> p t f", f=32)
                    wc = smpool.tile([128, B * 32], bf16, tag="wc")
                    wc3 = wc[:].rearrange("p (t f) -> p t f", f=32)
                    t2 = smpool.tile([128, B * 32], bf16, tag="t2")
                    t23 = t2[:].rearrange("p (t f) -> p t f", f=32)
                    tmp = smpool.tile([128, B * 32], bf16, tag="tmp")
                    tmp3 = tmp[:].rearrange("p (t f) -> p t f", f=32)
                    dot = smpool.tile([128, B * 32], bf16, tag="dot")
                    dot3 = dot[:].rearrange("p (t f) -> p t f", f=32)

                    ea0b = ea3[:, :, 0:1].to_broadcast([128, B, 32])
                    nc.vector.tensor_tensor(out=wa3[:], in0=w3v[:, :, 0:32],
                                            in1=ea0b, op=OP.mult)
                    nc.vector.tensor_tensor(out=wc3[:], in0=w3v[:, :, 64:96],
                                            in1=ea0b, op=OP.mult)
                    nc.vector.tensor_tensor(out=F3[:, :, 0:32], in0=wa3[:],
                                            in1=xs3[:, :, 0:32], op=OP.mult)
                    nc.vector.tensor_tensor(out=t23[:], in0=w3v[:, :, 32:64],
                                            in1=xs3[:, :, 0:32], op=OP.mult)
                    for d in range(3):
                        ea1b = ea3[:, :, 1 + d:2 + d].to_broadcast([128, B, 32])
                        x1sd = xs3[:, :, 32 + 32 * d:64 + 32 * d]
                        nc.vector.tensor_tensor(
                            out=F3[:, :, 96 + 64 * d:128 + 64 * d],
                            in0=wc3[:], in1=x1sd, op=OP.mult)
                        nc.vector.tensor_tensor(
                            out=F3[:, :, 64 + 64 * d:96 + 64 * d],
                            in0=t23[:], in1=ea1b, op=OP.mult)
                        if d == 0:
                            nc.vector.tensor_tensor(out=dot3[:], in0=x1sd,
                                                    in1=ea1b, op=OP.mult)
                        else:
                            nc.vector.tensor_tensor(out=tmp3[:], in0=x1sd,
                                                    in1=ea1b, op=OP.mult)
                            nc.vector.tensor_tensor(out=dot3[:], in0=dot3[:],
                                                    in1=tmp3[:], op=OP.add)
                    nc.vector.tensor_tensor(out=F3[:, :, 32:64],
                                            in0=w3v[:, :, 96:128],
                                            in1=dot3[:], op=OP.mult)
                    if stage < 6:
                        continue
                    oh = xspool.tile([128, EB], bf16, tag="oh")
                    dstb = dst_sb[:, b * B:(b + 1) * B].rearrange(
                        "p (t j) -> p t j", j=1).to_broadcast([128, B, 128])
                    nc.vector.tensor_tensor(
                        out=oh[:].rearrange("p (t j) -> p t j", j=128),
                        in0=iota_sb[:].rearrange("p (t j) -> p t j", j=128),
                        in1=dstb, op=OP.is_equal)
                    agg = ps_agg.tile([128, 256], f32, tag="agg")
                    for t in range(B):
                        nc.tensor.matmul(out=agg[:],
                                         lhsT=oh[:, t * 128:(t + 1) * 128],
                                         rhs=F[:, t * 256:(t + 1) * 256],
                                         start=(t == 0), stop=(t == B - 1))
                    agg_sb = smpool.tile([128, 256], f32, tag="aggsb")
                    nc.scalar.activation(agg_sb[:], agg[:], AF.Copy,
                                         scale=attrP_sb[:, b:b + 1])
                    if stage < 7:
                        continue
                    atp = ps_agg.tile([128, 256], f32, tag="agg")
                    nc.tensor.transpose(out=atp[:, 0:128], in_=agg_sb[:, 0:128],
                                        identity=iden_f[:])
                    nc.tensor.transpose(out=atp[:, 128:256],
                                        in_=agg_sb[:, 128:256],
                                        identity=iden_f[:])
                    aggT = smpool.tile([128, 256], f32, tag="aggT")
                    nc.vector.tensor_copy(aggT[:], atp[:])
                    if stage < 8:
                        continue
                    zT_sb = smpool.tile([32, 512], f32, tag="zTsb")
                    zw = [(Wz0_sb, 0), (WzH_sb, 0), (WzL_sb, 128), (WzH_sb, 128)]
                    for g4, (wsb, hoff) in enumerate(zw):
                        zps = ps_z.tile([32, 128], f32, tag="z")
                        nc.tensor.matmul(out=zps[:],
                                         lhsT=wsb[:],
                                         rhs=aggT[:, hoff:hoff + 128],
                                         start=True, stop=True)
                        nc.vector.tensor_copy(zT_sb[0:32, 128 * g4:128 * g4 + 128],
                                              zps[:])
                    if stage < 10:
                        continue
                    zb = ps_z.tile([128, 128], f32, tag="zb")
                    for g4 in range(4):
                        nc.tensor.transpose(
                            out=zb[:, 32 * g4:32 * g4 + 32],
                            in_=zT_sb[0:32, 128 * g4:128 * g4 + 128],
                            identity=iden_f[0:32, 0:32])
                    nc.vector.tensor_tensor(
                        out=o_sb[:, b * 128:(b + 1) * 128],
                        in0=s_sb[:, b * 128:(b + 1) * 128],
                        in1=zb[:], op=OP.add)
                nc.sync.dma_start(out_d[:], o_sb[:])

            if reps == 1:
                body()
            else:
                with tc.For_i(0, reps, 1):
                    body()

    nc.compile()
    return nc


def _make_runner(nc, n_cores=NCORES):
    import jax
    from jax.sharding import Mesh, PartitionSpec
    from jax.experimental.shard_map import shard_map
    import concourse.mybir as mybir
    from concourse.bass2jax import (_bass_exec_p, install_neuronx_cc_hook,
                                    partition_id_tensor)

    install_neuronx_cc_hook()
    in_names, out_names, out_avals = [], [], []
    partition_name = nc.partition_id_tensor.name if nc.partition_id_tensor else None
    for alloc in nc.m.functions[0].allocations:
        if not isinstance(alloc, mybir.MemoryLocationSet):
            continue
        name = alloc.memorylocations[0].name
        if alloc.kind == "ExternalInput":
            if name != partition_name:
                in_names.append(name)
        elif alloc.kind == "ExternalOutput":
            out_avals.append(jax.core.ShapedArray(
                tuple(alloc.tensor_shape), mybir.dt.np(alloc.dtype)))
            out_names.append(name)
    all_in_names = list(in_names) + list(out_names)
    if partition_name is not None:
        all_in_names.append(partition_name)

    def _body(*args):
        operands = list(args)
        if partition_name is not None:
            operands.append(partition_id_tensor())
        return tuple(_bass_exec_p.bind(
            *operands, out_avals=tuple(out_avals), in_names=tuple(all_in_names),
            out_names=tuple(out_names), lowering_input_output_aliases=(),
            sim_require_finite=True, sim_require_nnan=True, nc=nc))

    try:
        devices = jax.devices("axon")[:n_cores]
    except RuntimeError:
        devices = jax.devices()[:n_cores]
    mesh = Mesh(np.asarray(devices), ("core",))
    fn = jax.jit(
        shard_map(_body, mesh=mesh,
                  in_specs=(PartitionSpec("core"),) * (len(in_names) + len(out_names)),
                  out_specs=(PartitionSpec("core"),) * len(out_names),
                  check_rep=False),
        keep_unused=True)
    return fn, in_names, out_names, out_avals


def _core_inputs(k, cores, shared):
    c = cores[k]
    return {
        "xdT": shared["xdT"],
        "xdT_own": np.ascontiguousarray(
            shared["xdT"][:, k * NODES_PER_CORE:(k + 1) * NODES_PER_CORE]),
        "attrP": np.ascontiguousarray(
            shared["attrP"][:, k * NT_PER_CORE:(k + 1) * NT_PER_CORE]),
        "Wbig": shared["Wbig"], "Wscbig": shared["Wscbig"],
        "Wfc1": shared["Wfc1"], "Wfc2": shared["Wfc2"],
        "Wz0": shared["Wz0"], "WzL": shared["WzL"], "WzH": shared["WzH"],
        "srcidx": c["srcidx"], "dstloc": c["dstloc"],
        "eaP": c["eaP"], "ebT": c["ebT"],
    }


def _run(nc, cores, shared, n_cores=NCORES):
    import jax
    fn, in_names, out_names, out_avals = _make_runner(nc, n_cores)
    per_core = [_core_inputs(k, cores, shared) for k in range(n_cores)]
    concat_in = [np.concatenate([np.ascontiguousarray(per_core[c][n])
                                 for c in range(n_cores)], axis=0)
                 for n in in_names]
    zeros = [np.zeros((n_cores * a.shape[0], *a.shape[1:]), a.dtype)
             for a in out_avals]
    args = [jax.device_put(x) for x in concat_in + zeros]
    r = fn(*args)
    jax.block_until_ready(r)
    oi = out_names.index("outp")
    full = np.asarray(r[oi]).reshape(n_cores, 128, NODES_PER_CORE)
    return full, (fn, args)


def _unshard(full, N):
    # full [NCORES, 128part, 49*128]; node (k, b*128+p) at [k, p, b*128... ]
    # o_sb[:, b*128:(b+1)*128] holds out tile [node p, feat] transposed?  No:
    # o_sb column-block b is [128 nodes(part), 128 feat]?  o_sb is [128, 6272]
    # with block b = otile [128p, 128f] -> node n=b*128+p, feature f.
    out = np.zeros((N, 128), np.float32)
    k_all = full.reshape(NCORES, 128, NT_PER_CORE, 128).transpose(0, 2, 1, 3)
    flat = k_all.reshape(NCORES * NODES_PER_CORE, 128)[:N]
    out[:, :32] = flat[:, :32]
    out[:, 32:] = flat[:, 32:].reshape(N, 3, 32).transpose(0, 2, 1).reshape(N, 96)
    return out


_CACHE = {}


def kernel(**inputs):
    cores, shared, meta = _prep(inputs)
    key = ("k1", meta["B_max"])
    if key not in _CACHE:
        _CACHE[key] = _build_nc(meta, reps=1)
    nc = _CACHE[key]
    full, _ = _run(nc, cores, shared)
    return _unshard(full, meta["N"])



# revision 6
# speedup vs baseline: 80.8869x; 80.8869x over previous
"""Trainium2 Bass kernel for nn_Convolution_84172769067729 (e3nn-style GNN conv).

Graph/edge parallelism over 8 NeuronCores:
  - Node space padded to 50176 = 8*49*128; core k owns dst range
    [k*6272,(k+1)*6272). Host buckets edges by destination node-tile (128
    nodes), pads each (core,bucket) to a uniform B_max tiles of 128 edges
    (zero-embedding pad edges contribute exactly zero), and pre-permutes
    all per-edge arrays into that order.
  - Stage B: y = lin1(x*attr) into an HBM table (partition-major row
    mapping so table writes are large contiguous runs); self-connection s
    for the own range. Host folds all norm factors / c_s / c_x /
    1/sqrt(NUM_NEIGHBORS) into the weights.
  - Stage C per bucket: indirect-DMA gather y[src] (f32->bf16 cast in
    DMA), radial MLP (PE + ACT silu, bf16), tensor product as bf16 DVE
    slab ops, scatter-add via one-hot matmuls accumulated in PSUM, lin2 +
    combine, output written once as a [128, 6272] partition-major slab.
Host unshards: reorder columns (d-major -> u-major), un-permute rows.
"""
import math
import numpy as np
import ml_dtypes

MUL = 32
NCORES = 8
NODE_TILE = 128
NT_PER_CORE = 49
NODES_PER_CORE = NT_PER_CORE * NODE_TILE      # 6272
NN = NCORES * NODES_PER_CORE                  # 50176
NTILES = NN // 128                            # 392
NB = 10
NHID = 100

_BF16 = np.dtype(ml_dtypes.bfloat16)


def _prep(inputs):
    node_input = np.asarray(inputs["node_input"], np.float32)
    node_attr = np.asarray(inputs["node_attr"], np.float32)
    edge_src = np.asarray(inputs["edge_src"], np.int32)
    edge_dst = np.asarray(inputs["edge_dst"], np.int32)
    edge_attr = np.asarray(inputs["edge_attr"], np.float32)
    elemb = np.asarray(inputs["edge_length_embedded"], np.float32)
    N = node_input.shape[0]

    c_s = math.sin(math.pi / 8.0)
    c_x = math.cos(math.pi / 8.0)
    Wl10 = np.asarray(inputs["W_l10"], np.float32) / math.sqrt(MUL)
    Wl11 = np.asarray(inputs["W_l11"], np.float32) / math.sqrt(MUL)
    Wsc0 = np.asarray(inputs["W_sc0"], np.float32) * (c_s / math.sqrt(MUL))
    Wsc1 = np.asarray(inputs["W_sc1"], np.float32) * (c_s / math.sqrt(MUL))
    Wfc1 = np.asarray(inputs["W_fc1"], np.float32) / math.sqrt(NB)
    Wfc2 = (np.asarray(inputs["W_fc2"], np.float32) / math.sqrt(NHID)).copy()
    Wfc2[:, 3 * MUL:] /= math.sqrt(3.0)
    zf = c_x / math.sqrt(2 * MUL) / math.sqrt(16.0)
    Wl20 = np.asarray(inputs["W_l20"], np.float32) * zf
    Wl21 = np.asarray(inputs["W_l21"], np.float32) * zf
    # F column layout interleaves mid1a_d/mid1b_d as 64-wide [C_d|D_d] pairs.
    # lin2 weights zero-padded to K=128 so every z matmul reads a full
    # base-partition-0 aggT half (base!=0 operands crash the PE).
    Wz0 = np.zeros((128, 32), np.float32)
    Wz0[0:64] = Wl20
    WzL = np.zeros((128, 32), np.float32)
    WzL[0:64] = Wl21
    WzH = np.zeros((128, 32), np.float32)
    WzH[64:128] = Wl21

    Wbig = np.zeros((128, 128), np.float32)
    Wscbig = np.zeros((128, 128), np.float32)
    Wbig[0:32, 0:32] = Wl10
    Wscbig[0:32, 0:32] = Wsc0
    for d in range(3):
        sl = slice(32 + 32 * d, 64 + 32 * d)
        Wbig[sl, sl] = Wl11
        Wscbig[sl, sl] = Wsc1

    x0 = node_input[:, :MUL] * node_attr
    x1 = node_input[:, MUL:].reshape(N, MUL, 3) * node_attr[:, :, None]
    xdT = np.zeros((128, NN), np.float32)
    xdT[0:32, :N] = x0.T
    for d in range(3):
        xdT[32 + 32 * d:64 + 32 * d, :N] = x1[:, :, d].T

    attr = np.zeros(NN, np.float32)
    attr[:N] = node_attr[:, 0]
    attrP = attr.reshape(NTILES, 128).T.copy()      # [128, 392]

    core_of = edge_dst // NODES_PER_CORE
    bucket_of = (edge_dst % NODES_PER_CORE) // NODE_TILE
    counts = np.zeros((NCORES, NT_PER_CORE), np.int64)
    np.add.at(counts, (core_of, bucket_of), 1)
    B_max = int(np.ceil(counts.max() / 128))
    T = NT_PER_CORE * B_max
    Epc = T * 128

    order = np.argsort(edge_dst, kind="stable")
    es, ed = edge_src[order], edge_dst[order]
    ea_s, eb_s = edge_attr[order], elemb[order]
    flat_bucket = (edge_dst[order] // NODE_TILE)
    starts = np.searchsorted(flat_bucket, np.arange(NCORES * NT_PER_CORE))
    ends = np.searchsorted(flat_bucket, np.arange(NCORES * NT_PER_CORE) + 1)

    cores = []
    for k in range(NCORES):
        srcidx = np.zeros(Epc, np.int32)
        dstloc = np.zeros(Epc, np.float32)
        ea = np.zeros((Epc, 4), np.float32)
        eb = np.zeros((Epc, NB), np.float32)
        for b in range(NT_PER_CORE):
            g = k * NT_PER_CORE + b
            i0, i1 = starts[g], ends[g]
            n = i1 - i0
            o = b * B_max * 128
            srcidx[o:o + n] = es[i0:i1]
            dstloc[o:o + n] = (ed[i0:i1] % NODE_TILE).astype(np.float32)
            ea[o:o + n] = ea_s[i0:i1]
            eb[o:o + n] = eb_s[i0:i1]
        # partition-major ytab row mapping: node n -> row (n%128)*392 + n//128
        srcmap = (srcidx % 128) * NTILES + srcidx // 128
        cores.append({
            "srcidx": np.ascontiguousarray(srcmap.reshape(T, 128).T),
            "dstloc": np.ascontiguousarray(dstloc.reshape(T, 128).T).astype(_BF16),
            "eaP": np.ascontiguousarray(
                ea.reshape(T, 128, 4).transpose(1, 0, 2).reshape(128, T * 4)
            ).astype(_BF16),
            "ebT": np.ascontiguousarray(eb.T).astype(_BF16),
        })
    shared = {
        "xdT": xdT.astype(_BF16), "xdTo_f32": xdT, "attrP": attrP,
        "Wbig": Wbig.astype(_BF16), "Wscbig": Wscbig,
        "Wfc1": Wfc1.astype(_BF16), "Wfc2": Wfc2.astype(_BF16),
        "Wz0": Wz0, "WzL": WzL, "WzH": WzH,
    }
    meta = {"B_max": B_max, "T": T, "Epc": Epc, "N": N}
    return cores, shared, meta


def _build_nc(meta, reps=1, stage=10):
    import concourse.bacc as bacc
    import concourse.bass as bass
    import concourse.mybir as mybir
    import concourse.tile as tile

    f32 = mybir.dt.float32
    bf16 = mybir.dt.bfloat16
    i32 = mybir.dt.int32
    i16 = mybir.dt.int16
    AF = mybir.ActivationFunctionType
    OP = mybir.AluOpType
    B = meta["B_max"]
    T = meta["T"]
    Epc = meta["Epc"]
    EB = B * 128

    nc = bacc.Bacc("TRN2", debug=False, num_devices=NCORES)

    xdT_d = nc.dram_tensor("xdT", [128, NN], bf16, kind="ExternalInput")
    xdTo_d = nc.dram_tensor("xdT_own", [128, NODES_PER_CORE], f32, kind="ExternalInput")
    attrP_d = nc.dram_tensor("attrP", [128, NT_PER_CORE], f32, kind="ExternalInput")
    Wbig_d = nc.dram_tensor("Wbig", [128, 128], bf16, kind="ExternalInput")
    Wscbig_d = nc.dram_tensor("Wscbig", [128, 128], f32, kind="ExternalInput")
    Wfc1_d = nc.dram_tensor("Wfc1", [NB, NHID], bf16, kind="ExternalInput")
    Wfc2_d = nc.dram_tensor("Wfc2", [NHID, 128], bf16, kind="ExternalInput")
    Wz0_d = nc.dram_tensor("Wz0", [128, 32], f32, kind="ExternalInput")
    WzL_d = nc.dram_tensor("WzL", [128, 32], f32, kind="ExternalInput")
    WzH_d = nc.dram_tensor("WzH", [128, 32], f32, kind="ExternalInput")
    src_d = nc.dram_tensor("srcidx", [128, T], i32, kind="ExternalInput")
    dst_d = nc.dram_tensor("dstloc", [128, T], bf16, kind="ExternalInput")
    eaP_d = nc.dram_tensor("eaP", [128, T * 4], bf16, kind="ExternalInput")
    ebT_d = nc.dram_tensor("ebT", [NB, Epc], bf16, kind="ExternalInput")
    ytab = nc.dram_tensor("ytab", [NN, 128], bf16)
    out_d = nc.dram_tensor("outp", [128, NT_PER_CORE * 128], f32, kind="ExternalOutput")

    with tile.TileContext(nc) as tc:
        with (
            tc.tile_pool(name="const", bufs=1) as cpool,
            tc.tile_pool(name="xslab", bufs=2) as xpool,
            tc.tile_pool(name="ysl", bufs=2) as ypool,
            tc.tile_pool(name="persist", bufs=1) as ppool,
            tc.tile_pool(name="eb", bufs=2) as ebpool,
            tc.tile_pool(name="xs", bufs=2) as xspool,
            tc.tile_pool(name="work", bufs=2) as wpool,
            tc.tile_pool(name="small", bufs=2) as smpool,
            tc.tile_pool(name="ps_big", bufs=2, space="PSUM") as ps_big,
            tc.tile_pool(name="ps_agg", bufs=2, space="PSUM") as ps_agg,
            tc.tile_pool(name="ps_z", bufs=2, space="PSUM") as ps_z,
        ):
            Wbig_sb = cpool.tile([128, 128], bf16)
            nc.sync.dma_start(Wbig_sb[:], Wbig_d[:])
            Wscbig_sb = cpool.tile([128, 128], f32)
            nc.sync.dma_start(Wscbig_sb[:], Wscbig_d[:])
            Wfc1_sb = cpool.tile([NB, NHID], bf16)
            nc.sync.dma_start(Wfc1_sb[:], Wfc1_d[:])
            Wfc2_sb = cpool.tile([NHID, 128], bf16)
            nc.sync.dma_start(Wfc2_sb[:], Wfc2_d[:])
            Wz0_sb = cpool.tile([128, 32], f32)
            nc.sync.dma_start(Wz0_sb[:], Wz0_d[:])
            WzL_sb = cpool.tile([128, 32], f32)
            nc.sync.dma_start(WzL_sb[:], WzL_d[:])
            WzH_sb = cpool.tile([128, 32], f32)
            nc.sync.dma_start(WzH_sb[:], WzH_d[:])
            attrP_sb = cpool.tile([128, NT_PER_CORE], f32)
            nc.sync.dma_start(attrP_sb[:], attrP_d[:])
            src_sb = cpool.tile([128, T], i32)
            nc.sync.dma_start(src_sb[:], src_d[:])
            dst_sb = cpool.tile([128, T], bf16)
            nc.sync.dma_start(dst_sb[:], dst_d[:])
            eaP_sb = cpool.tile([128, T * 4], bf16)
            nc.sync.dma_start(eaP_sb[:], eaP_d[:])

            iota_i = cpool.tile([128, EB], i16)
            nc.gpsimd.iota(iota_i[:].rearrange("p (t j) -> p t j", j=128),
                           pattern=[[0, B], [1, 128]], channel_multiplier=0)
            iota_sb = cpool.tile([128, EB], bf16)
            nc.vector.tensor_copy(iota_sb[:], iota_i[:])

            iden_i = cpool.tile([128, 128], i16)
            nc.gpsimd.iota(iden_i[:], pattern=[[1, 128]], channel_multiplier=0)
            iden_t = cpool.tile([128, 128], f32)
            nc.vector.tensor_copy(iden_t[:], iden_i[:])
            pidx_i = cpool.tile([128, 1], i16)
            nc.gpsimd.iota(pidx_i[:], pattern=[[0, 1]], channel_multiplier=1)
            pidx_f = cpool.tile([128, 1], f32)
            nc.vector.tensor_copy(pidx_f[:], pidx_i[:])
            iden_f = cpool.tile([128, 128], f32)
            nc.vector.tensor_scalar(out=iden_f[:], in0=iden_t[:],
                                    scalar1=pidx_f[:, 0:1], scalar2=None,
                                    op0=OP.is_equal)

            s_sb = ppool.tile([128, NODES_PER_CORE], f32)
            o_sb = ppool.tile([128, NODES_PER_CORE], f32)
            if stage < 10:
                nc.vector.memset(o_sb[:], 0)
            ytv = ytab[:].rearrange("(p t) f -> p t f", t=NTILES)

            def body():
                # ---------- stage B ----------
                SL = 16
                for s0 in range(0, NTILES, SL):
                    nsl = min(SL, NTILES - s0)
                    xsl = xpool.tile([128, SL * 128], bf16, tag="xsl")
                    nc.sync.dma_start(xsl[:, :nsl * 128],
                                      xdT_d[:, s0 * 128:(s0 + nsl) * 128])
                    ysl = ypool.tile([128, SL * 128], bf16, tag="ysl")
                    for g in range(0, nsl, 4):
                        nq = min(4, nsl - g)
                        yps = ps_big.tile([128, 512], f32, tag="big")
                        for q in range(nq):
                            nc.tensor.matmul(
                                out=yps[:, q * 128:(q + 1) * 128],
                                lhsT=xsl[:, (g + q) * 128:(g + q + 1) * 128],
                                rhs=Wbig_sb[:], start=True, stop=True)
                        nc.scalar.activation(ysl[:, g * 128:(g + nq) * 128],
                                             yps[:, :nq * 128], AF.Copy)
                    nc.sync.dma_start(
                        ytv[:, s0:s0 + nsl, :],
                        ysl[:, :nsl * 128].rearrange("p (q f) -> p q f", f=128))
                for b0 in range(0, NT_PER_CORE, 4):
                    nb4 = min(4, NT_PER_CORE - b0)
                    xso = xpool.tile([128, 4 * 128], f32, tag="xso")
                    nc.sync.dma_start(xso[:, :nb4 * 128],
                                      xdTo_d[:, b0 * 128:(b0 + nb4) * 128])
                    sps = ps_big.tile([128, 512], f32, tag="big")
                    for q in range(nb4):
                        nc.tensor.matmul(out=sps[:, q * 128:(q + 1) * 128],
                                         lhsT=xso[:, q * 128:(q + 1) * 128],
                                         rhs=Wscbig_sb[:], start=True, stop=True)
                    nc.scalar.activation(s_sb[:, b0 * 128:(b0 + nb4) * 128],
                                         sps[:, :nb4 * 128], AF.Copy)

                # ---------- stage C ----------
                for b in range(NT_PER_CORE if stage >= 2 else 0):
                    e0 = b * EB
                    ebsl = ebpool.tile([NB, EB], bf16, tag="ebsl")
                    nc.sync.dma_start(ebsl[:], ebT_d[:, e0:e0 + EB])
                    xs = xspool.tile([128, EB], bf16, tag="xs")
                    for t in range(B):
                        nc.gpsimd.indirect_dma_start(
                            out=xs[:, t * 128:(t + 1) * 128],
                            out_offset=None,
                            in_=ytab[:],
                            in_offset=bass.IndirectOffsetOnAxis(
                                ap=src_sb[:, b * B + t:b * B + t + 1], axis=0),
                        )
                    if stage < 3:
                        continue
                    hT = wpool.tile([NHID, EB], bf16, tag="hT")
                    for c0 in range(0, EB, 512):
                        cw = min(512, EB - c0)
                        hps = ps_big.tile([NHID, 512], f32, tag="big")
                        nc.tensor.matmul(out=hps[:, :cw], lhsT=Wfc1_sb[:],
                                         rhs=ebsl[:, c0:c0 + cw],
                                         start=True, stop=True)
                        nc.scalar.activation(hT[:, c0:c0 + cw], hps[:, :cw],
                                             AF.Silu)
                    if stage < 4:
                        continue
                    wsl = wpool.tile([128, EB], bf16, tag="wsl")
                    for t0 in range(0, B, 4):
                        nt4 = min(4, B - t0)
                        wps = ps_big.tile([128, 512], f32, tag="big")
                        for q in range(nt4):
                            t = t0 + q
                            nc.tensor.matmul(out=wps[:, q * 128:(q + 1) * 128],
                                             lhsT=hT[:, t * 128:(t + 1) * 128],
                                             rhs=Wfc2_sb[:], start=True, stop=True)
                        nc.vector.tensor_copy(wsl[:, t0 * 128:(t0 + nt4) * 128],
                                              wps[:, :nt4 * 128])

                    if stage < 5:
                        continue
                    xs3 = xs[:].rearrange("p (t f) -> p t f", f=128)
                    w3v = wsl[:].rearrange("p (t f) -> p t f", f=128)
                    ea3 = eaP_sb[:, b * B * 4:(b + 1) * B * 4].rearrange(
                        "p (t j) -> p t j", j=4)
                    F = wpool.tile([128, B * 256], bf16, tag="F")
                    F3 = F[:].rearrange("p (t f) -> p t f", f=256)
                    wa = smpool.tile([128, B * 32], bf16, tag="wa")
                    wa3 = wa[:].rearrange("p (t f) -> p t f", f=32)
                    wc = smpool.tile([128, B * 32], bf16, tag="wc")
                    wc3 = wc[:].rearrange("p (t f) -> p t f", f=32)
                    t2 = smpool.tile([128, B * 32], bf16, tag="t2")
                    t23 = t2[:].rearrange("p (t f) -> p t f", f=32)
                    tmp = smpool.tile([128, B * 32], bf16, tag="tmp")
                    tmp3 = tmp[:].rearrange("p (t f) -> p t f", f=32)
                    dot = smpool.tile([128, B * 32], bf16, tag="dot")
                    dot3 = dot[:].rearrange("p (t f) -> p t f", f=32)

                    ea0b = ea3[:, :, 0:1].to_broadcast([128, B, 32])
                    nc.vector.tensor_tensor(out=wa3[:], in0=w3v[:, :, 0:32],
                                            in1=ea0b, op=OP.mult)
                    nc.vector.tensor_tensor(out=wc3[:], in0=w3v[:, :, 64:96],
                                            in1=ea0b, op=OP.mult)
                    nc.vector.tensor_tensor(out=F3[:, :, 0:32], in0=wa3[:],
                                            in1=xs3[:, :, 0:32], op=OP.mult)
                    nc.vector.tensor_tensor(out=t23[:], in0=w3v[:, :, 32:64],
                                            in1=xs3[:, :, 0:32], op=OP.mult)
                    for d in range(3):
                        ea1b = ea3[:, :, 1 + d:2 + d].to_broadcast([128, B, 32])
                        x1sd = xs3[:, :, 32 + 32 * d:64 + 32 * d]
                        nc.vector.tensor_tensor(
                            out=F3[:, :, 96 + 64 * d:128 + 64 * d],
                            in0=wc3[:], in1=x1sd, op=OP.mult)
                        nc.vector.tensor_tensor(
                            out=F3[:, :, 64 + 64 * d:96 + 64 * d],
                            in0=t23[:], in1=ea1b, op=OP.mult)
                        if d == 0:
                            nc.vector.tensor_tensor(out=dot3[:], in0=x1sd,
                                                    in1=ea1b, op=OP.mult)
                        else:
                            nc.vector.tensor_tensor(out=tmp3[:], in0=x1sd,
                                                    in1=ea1b, op=OP.mult)
                            nc.vector.tensor_tensor(out=dot3[:], in0=dot3[:],
                                                    in1=tmp3[:], op=OP.add)
                    nc.vector.tensor_tensor(out=F3[:, :, 32:64],
                                            in0=w3v[:, :, 96:128],
                                            in1=dot3[:], op=OP.mult)
                    if stage < 6:
                        continue
                    oh = xspool.tile([128, EB], bf16, tag="oh")
                    dstb = dst_sb[:, b * B:(b + 1) * B].rearrange(
                        "p (t j) -> p t j", j=1).to_broadcast([128, B, 128])
                    nc.vector.tensor_tensor(
                        out=oh[:].rearrange("p (t j) -> p t j", j=128),
                        in0=iota_sb[:].rearrange("p (t j) -> p t j", j=128),
                        in1=dstb, op=OP.is_equal)
                    agg = ps_agg.tile([128, 256], f32, tag="agg")
                    for t in range(B):
                        nc.tensor.matmul(out=agg[:],
                                         lhsT=oh[:, t * 128:(t + 1) * 128],
                                         rhs=F[:, t * 256:(t + 1) * 256],
                                         start=(t == 0), stop=(t == B - 1))
                    agg_sb = smpool.tile([128, 256], f32, tag="aggsb")
                    nc.scalar.activation(agg_sb[:], agg[:], AF.Copy,
                                         scale=attrP_sb[:, b:b + 1])
                    if stage < 7:
                        continue
                    atp = ps_agg.tile([128, 256], f32, tag="agg")
                    nc.tensor.transpose(out=atp[:, 0:128], in_=agg_sb[:, 0:128],
                                        identity=iden_f[:])
                    nc.tensor.transpose(out=atp[:, 128:256],
                                        in_=agg_sb[:, 128:256],
                                        identity=iden_f[:])
                    aggT = smpool.tile([128, 256], f32, tag="aggT")
                    nc.vector.tensor_copy(aggT[:], atp[:])
                    if stage < 8:
                        continue
                    zT_sb = smpool.tile([32, 512], f32, tag="zTsb")
                    zw = [(Wz0_sb, 0), (WzH_sb, 0), (WzL_sb, 128), (WzH_sb, 128)]
                    for g4, (wsb, hoff) in enumerate(zw):
                        zps = ps_z.tile([32, 128], f32, tag="z")
                        nc.tensor.matmul(out=zps[:],
                                         lhsT=wsb[:],
                                         rhs=aggT[:, hoff:hoff + 128],
                                         start=True, stop=True)
                        nc.vector.tensor_copy(zT_sb[0:32, 128 * g4:128 * g4 + 128],
                                              zps[:])
                    if stage < 10:
                        continue
                    zb = ps_z.tile([128, 128], f32, tag="zb")
                    for g4 in range(4):
                        nc.tensor.transpose(
                            out=zb[:, 32 * g4:32 * g4 + 32],
                            in_=zT_sb[0:32, 128 * g4:128 * g4 + 128],
                            identity=iden_f[0:32, 0:32])
                    nc.vector.tensor_tensor(
                        out=o_sb[:, b * 128:(b + 1) * 128],
                        in0=s_sb[:, b * 128:(b + 1) * 128],
                        in1=zb[:], op=OP.add)
                nc.sync.dma_start(out_d[:], o_sb[:])

            if reps == 1:
                body()
            else:
                with tc.For_i(0, reps, 1):
                    body()

    nc.compile()
    return nc


def _make_runner(nc, n_cores=NCORES):
    import jax
    from jax.sharding import Mesh, PartitionSpec
    from jax.experimental.shard_map import shard_map
    import concourse.mybir as mybir
    from concourse.bass2jax import (_bass_exec_p, install_neuronx_cc_hook,
                                    partition_id_tensor)

    install_neuronx_cc_hook()
    in_names, out_names, out_avals = [], [], []
    partition_name = nc.partition_id_tensor.name if nc.partition_id_tensor else None
    for alloc in nc.m.functions[0].allocations:
        if not isinstance(alloc, mybir.MemoryLocationSet):
            continue
        name = alloc.memorylocations[0].name
        if alloc.kind == "ExternalInput":
            if name != partition_name:
                in_names.append(name)
        elif alloc.kind == "ExternalOutput":
            out_avals.append(jax.core.ShapedArray(
                tuple(alloc.tensor_shape), mybir.dt.np(alloc.dtype)))
            out_names.append(name)
    all_in_names = list(in_names) + list(out_names)
    if partition_name is not None:
        all_in_names.append(partition_name)

    def _body(*args):
        operands = list(args)
        if partition_name is not None:
            operands.append(partition_id_tensor())
        return tuple(_bass_exec_p.bind(
            *operands, out_avals=tuple(out_avals), in_names=tuple(all_in_names),
            out_names=tuple(out_names), lowering_input_output_aliases=(),
            sim_require_finite=True, sim_require_nnan=True, nc=nc))

    try:
        devices = jax.devices("axon")[:n_cores]
    except RuntimeError:
        devices = jax.devices()[:n_cores]
    mesh = Mesh(np.asarray(devices), ("core",))
    fn = jax.jit(
        shard_map(_body, mesh=mesh,
                  in_specs=(PartitionSpec("core"),) * (len(in_names) + len(out_names)),
                  out_specs=(PartitionSpec("core"),) * len(out_names),
                  check_rep=False),
        keep_unused=True)
    return fn, in_names, out_names, out_avals


def _core_inputs(k, cores, shared):
    c = cores[k]
    return {
        "xdT": shared["xdT"],
        "xdT_own": np.ascontiguousarray(
            shared["xdTo_f32"][:, k * NODES_PER_CORE:(k + 1) * NODES_PER_CORE]),
        "attrP": np.ascontiguousarray(
            shared["attrP"][:, k * NT_PER_CORE:(k + 1) * NT_PER_CORE]),
        "Wbig": shared["Wbig"], "Wscbig": shared["Wscbig"],
        "Wfc1": shared["Wfc1"], "Wfc2": shared["Wfc2"],
        "Wz0": shared["Wz0"], "WzL": shared["WzL"], "WzH": shared["WzH"],
        "srcidx": c["srcidx"], "dstloc": c["dstloc"],
        "eaP": c["eaP"], "ebT": c["ebT"],
    }


def _run(nc, cores, shared, n_cores=NCORES):
    import jax
    fn, in_names, out_names, out_avals = _make_runner(nc, n_cores)
    per_core = [_core_inputs(k, cores, shared) for k in range(n_cores)]
    concat_in = [np.concatenate([np.ascontiguousarray(per_core[c][n])
                                 for c in range(n_cores)], axis=0)
                 for n in in_names]
    zeros = [np.zeros((n_cores * a.shape[0], *a.shape[1:]), a.dtype)
             for a in out_avals]
    args = [jax.device_put(x) for x in concat_in + zeros]
    r = fn(*args)
    jax.block_until_ready(r)
    oi = out_names.index("outp")
    full = np.asarray(r[oi]).reshape(n_cores, 128, NODES_PER_CORE)
    return full, (fn, args)


def _unshard(full, N):
    # full [NCORES, 128part, 49*128]; node (k, b*128+p) at [k, p, b*128... ]
    # o_sb[:, b*128:(b+1)*128] holds out tile [node p, feat] transposed?  No:
    # o_sb column-block b is [128 nodes(part), 128 feat]?  o_sb is [128, 6272]
    # with block b = otile [128p, 128f] -> node n=b*128+p, feature f.
    out = np.zeros((N, 128), np.float32)
    k_all = full.reshape(NCORES, 128, NT_PER_CORE, 128).transpose(0, 2, 1, 3)
    flat = k_all.reshape(NCORES * NODES_PER_CORE, 128)[:N]
    out[:, :32] = flat[:, :32]
    out[:, 32:] = flat[:, 32:].reshape(N, 3, 32).transpose(0, 2, 1).reshape(N, 96)
    return out


_CACHE = {}


def kernel(**inputs):
    cores, shared, meta = _prep(inputs)
    key = ("k3", meta["B_max"])
    if key not in _CACHE:
        _CACHE[key] = _build_nc(meta, reps=1)
    nc = _CACHE[key]
    full, _ = _run(nc, cores, shared)
    return _unshard(full, meta["N"])

